# revision 16
# baseline (speedup 1.0000x reference)
"""GCN layer (gather + segment-sum + degree-normalize + linear) on 8 Trainium2 cores.

Strategy (v2)
-------------
Destination-node sharding: core k owns dest rows [k*D, (k+1)*D), D = n_nodes/8.
The host groups each core's edges by 128-dest windows (dest-sorted); the
on-device segment-sum is done per 128-edge chunk with a PE matmul in the
TRANSPOSED orientation: aggT[feat, dest] += G_chunk.T @ S_chunk, where
G_chunk = gathered bf16 source rows [128 edge, 128 feat] (stationary) and
S_chunk[e, j] = (col_rel[e] == j) built on DVE in bf16 (streaming rhs).

Per window: hT = aggT * recip_deg (Scalar engine, PSUM->SBUF, bf16 out),
outT = wt.T-matmul(lhsT=wt bf16, rhs=hT) + biasT (DVE add, PSUM->SBUF),
DMA outT column block. Degree reciprocals are computed on the HOST (pure
index preprocessing) and shipped replicated across partitions; the output
is produced transposed [out_f, dests] and transposed back on the host.

Measured facts driving the design:
- dma_gather is Q7 descriptor-emission bound at ~8 ns/desc/queue (4 queues);
  256B bf16 elements leave the 16 SDMA engines ~50% idle, so all compute
  must hide behind the gather. Payload halving (512B->256B hi/lo split
  removal) only helps ~18%; descriptor count is what matters.
- The v1 per-window f32 epilogue (DVE tensor_scalar/reciprocal + PE f32
  transpose + f32 W matmul) added ~120us to the critical path and its f32
  SBUF->SBUF DVE ops lock GPSIMD out of SWDGE descriptor rings.

Why not dma_scatter_add per edge: measured on HW, its read-modify-write races
lose updates whenever a destination index appears more than once per call.

dma_gather facts (measured): idx arrays are int16, wrapped [16, N/16] and
replicated into all eight 16-partition groups; single_packet=False is required
for calls over 1024 indices. int16 limits a gather call's index range to
32768 rows, so edges are split into lo/hi source streams gathered from base
x[0] / x[32768].
"""
import sys
import os
sys.path.insert(0, "/opt/trn_rl_repo")

import numpy as np

P = 128
GATHER_SPLIT = 25000       # lo/hi split at n/2: both halves < 32768 (int16
                           # gather idx) and balanced so per-stream queue
                           # pinning splits Q7 desc-gen work evenly
DEFAULT_BLK_CHUNKS = 16    # gather block/call size in 128-edge chunks (2048 idxs)
SBATCH = 8                 # S-matrix build batch, in chunks
N_CORES = 8


def _ceil_div(a, b):
    return -(-a // b)


def _wrap_idx(ix):
    """[N] int16 -> [128, N/16], idx i at [i%16, i//16], replicated into the
    eight 16-partition groups (the tx/rx Q7 cpus of every SWDGE queue each
    read their own group)."""
    n = len(ix)
    assert n % 16 == 0
    w = np.zeros((P, n // 16), np.int16)
    blk = ix.reshape(-1, 16).T
    for g in range(8):
        w[16 * g:16 * (g + 1), :] = blk
    return w


class Plan:
    """Host-side sharding: per-core per-stream edge arrays with a chunk
    structure (windows x chunk counts) identical across cores, so a single
    SPMD program serves all cores."""

    def __init__(self, row, col, n_nodes, n_cores=N_CORES,
                 blk_chunks=None, gather_split=GATHER_SPLIT):
        if blk_chunks is None:
            blk_chunks = int(os.environ.get("K_BLK", DEFAULT_BLK_CHUNKS))
        assert n_nodes % n_cores == 0
        self.n_cores = n_cores
        self.n_nodes = n_nodes
        self.d_core = n_nodes // n_cores
        self.n_win = _ceil_div(self.d_core, P)
        self.blk_chunks = blk_chunks
        self.gather_split = gather_split

        order = np.argsort(col, kind="stable")
        rs = row[order]
        cs = col[order]
        bounds = np.searchsorted(cs, np.arange(n_cores + 1) * self.d_core)

        W = self.n_win
        per_core = []  # [k][stream] = (rows, local_cols, per-window counts)
        cnt = {"lo": np.zeros(W, np.int64), "hi": np.zeros(W, np.int64)}
        for k in range(n_cores):
            a, b = bounds[k], bounds[k + 1]
            r_k = rs[a:b]
            lc_k = cs[a:b] - k * self.d_core
            lo = r_k < gather_split
            streams = {}
            for sname, mask in (("lo", lo), ("hi", ~lo)):
                r_s = r_k[mask]
                lc_s = lc_k[mask]
                counts = np.bincount(lc_s >> 7, minlength=W)
                streams[sname] = (r_s, lc_s, counts)
                cnt[sname] = np.maximum(cnt[sname], -(-counts // P))
            per_core.append(streams)
        cnt["lo"] = np.maximum(cnt["lo"], 1)  # every window gets >=1 chunk
        self.cnt = cnt
        self.off = {s: np.concatenate([[0], np.cumsum(cnt[s])]) for s in cnt}
        self.Csum = {s: int(self.off[s][-1]) for s in cnt}
        self.NB = {s: _ceil_div(self.Csum[s], blk_chunks) for s in cnt}
        self.Npad = {s: self.NB[s] * blk_chunks * P for s in cnt}

        # per-core degree reciprocals over local dests (host-side bincount)
        self.recip = []
        for k in range(n_cores):
            a, b = bounds[k], bounds[k + 1]
            lc_k = cs[a:b] - k * self.d_core
            deg = np.bincount(lc_k, minlength=W * P).astype(np.float32)
            self.recip.append(1.0 / np.maximum(deg, 1.0))

        self.core_arrays = []
        for k in range(n_cores):
            arrs = {}
            for sname in ("lo", "hi"):
                r_s, lc_s, counts = per_core[k][sname]
                off = self.off[sname]
                base = 0 if sname == "lo" else gather_split
                gidx = np.zeros(self.Npad[sname], np.int16)
                crel = np.full(self.Csum[sname] * P, -1, np.int8)
                if len(r_s):
                    starts = np.concatenate([[0], np.cumsum(counts)])
                    adj = off[:-1] * P - starts[:-1]
                    dst = np.arange(len(r_s)) + adj[lc_s >> 7]
                    gidx[dst] = (r_s - base).astype(np.int16)
                    crel[dst] = (lc_s & 127).astype(np.int8)
                arrs[f"gidx_{sname}"] = _wrap_idx(gidx)
                arrs[f"crel_{sname}"] = np.ascontiguousarray(
                    crel.reshape(self.Csum[sname], P).T)
            self.core_arrays.append(arrs)

    @property
    def total_chunks(self):
        return self.Csum["lo"] + self.Csum["hi"]


def _patch_swdge_lane_by_queue():
    """Pin each dma_gather's DMASW semaphore lane to its SWDGE queue number.

    Tile assigns DMASW lanes round-robin in scheduled order, which breaks when
    instructions on different queues (whose completions are only FIFO within a
    queue) share a lane. One lane per queue keeps per-lane completion in-order
    and lets gathers on the 4 queues run concurrently.
    """
    import concourse.tile_sem_assignment as tsa
    from concourse import mybir
    if getattr(tsa.TileClockTick, "_lane_by_queue_patch", False):
        return
    orig = tsa.TileClockTick._assign_tick

    def patched(self, inst):
        if isinstance(inst, mybir.InstDMAGatherAnt):
            if not hasattr(self, "_q_lane_ctr"):
                self._q_lane_ctr = {}
            q = inst.queue_num
            n = self._q_lane_ctr.get(q, 0)
            self._q_lane_ctr[q] = n + 1
            saved = self.next_sw_dma_idx
            # two lanes per queue: same-queue calls alternate lanes so a
            # call's desc-gen overlaps the previous call's DMA completion
            # (per-queue completion is FIFO, so lane ordering stays valid)
            self.next_sw_dma_idx = q * 2 + (n % 2)
            try:
                return orig(self, inst)
            finally:
                self.next_sw_dma_idx = saved
        return orig(self, inst)

    tsa.TileClockTick._assign_tick = patched
    tsa.TileClockTick._lane_by_queue_patch = True


def build_program(plan, in_f, out_f):
    """Emit the SPMD Bass program (shared by all cores)."""
    from concourse import bacc, mybir
    import concourse.tile as tile
    from contextlib import ExitStack

    _patch_swdge_lane_by_queue()
    skip_gather = os.environ.get("K_SKIP_GATHER") == "1"
    skip_compute = os.environ.get("K_SKIP_COMPUTE") == "1"

    f32 = mybir.dt.float32
    i16 = mybir.dt.int16
    i8 = mybir.dt.int8
    bf16 = mybir.dt.bfloat16

    W = plan.n_win
    BLK = plan.blk_chunks

    nc = bacc.Bacc("TRN2", target_bir_lowering=False, debug=False,
                   num_devices=plan.n_cores, num_swdge_queues=4)

    x_d = nc.dram_tensor("xb", [plan.n_nodes, in_f], bf16,
                         kind="ExternalInput")
    wt_d = nc.dram_tensor("wt", [in_f, out_f], bf16, kind="ExternalInput")
    biasT_d = nc.dram_tensor("biasT", [P, 1], f32, kind="ExternalInput")
    iota_d = nc.dram_tensor("iota", [P, P], bf16, kind="ExternalInput")
    recip_d = nc.dram_tensor("recip", [P, W * P], f32, kind="ExternalInput")
    gidx_d, crel_d = {}, {}
    for s in ("lo", "hi"):
        if plan.Csum[s] == 0:
            continue
        gidx_d[s] = nc.dram_tensor(f"gidx_{s}", [P, plan.Npad[s] // 16], i16,
                                   kind="ExternalInput")
        crel_d[s] = nc.dram_tensor(f"crel_{s}", [P, plan.Csum[s]], i8,
                                   kind="ExternalInput")
    outT_d = nc.dram_tensor("outT", [out_f, W * P], f32, kind="ExternalOutput")

    x_base = {"lo": x_d[:], "hi": x_d[plan.gather_split:, :]}

    with tile.TileContext(nc) as tc, ExitStack() as ctx:
        cpool = ctx.enter_context(tc.tile_pool(name="const", bufs=1))
        gpool = {s: ctx.enter_context(tc.tile_pool(name=f"g_{s}", bufs=2))
                 for s in ("lo", "hi")}
        spool = {s: ctx.enter_context(tc.tile_pool(name=f"s_{s}", bufs=3))
                 for s in ("lo", "hi")}
        epool = ctx.enter_context(tc.tile_pool(name="epi", bufs=3))
        apool = ctx.enter_context(tc.tile_pool(name="psum_a", bufs=4,
                                               space="PSUM"))
        opool = ctx.enter_context(tc.tile_pool(name="psum_o", bufs=2,
                                               space="PSUM"))

        # ---- constants ----
        # Sync ring: gidx block slices only, interleaved lo/hi in gather
        # issue order (the first gather of each stream waits only on its own
        # slice). Scalar ring: S-build inputs first (iota, crel), then the
        # first recip slice (window-0 epilogue), wt/bias, bulk recip.
        gidx_t, crel_f = {}, {}
        git = {}
        for s in ("lo", "hi"):
            if plan.Csum[s] == 0:
                continue
            git[s] = cpool.tile([P, plan.Npad[s] // 16], i16, name=f"gidx{s}")
            gidx_t[s] = git[s]
        for i in range(max(plan.NB.values())):
            for s in ("lo", "hi"):
                if plan.Csum[s] == 0 or i >= plan.NB[s]:
                    continue
                ncol = plan.Npad[s] // 16
                step = _ceil_div(ncol, plan.NB[s])
                c0, c1 = i * step, min((i + 1) * step, ncol)
                if c0 < c1:
                    nc.sync.dma_start(out=git[s][:, c0:c1],
                                      in_=gidx_d[s][:, c0:c1])
        iota_t = cpool.tile([P, P], bf16)
        nc.scalar.dma_start(out=iota_t[:], in_=iota_d[:])
        for s in ("lo", "hi"):
            if plan.Csum[s] == 0:
                continue
            cri = cpool.tile([P, plan.Csum[s]], i8, name=f"creli{s}")
            nc.scalar.dma_start(out=cri[:], in_=crel_d[s][:])
            crf = cpool.tile([P, plan.Csum[s]], bf16, name=f"crelf{s}")
            nc.vector.tensor_copy(out=crf[:], in_=cri[:])
            crel_f[s] = crf
        recip_t = cpool.tile([P, W * P], f32)
        rstep = _ceil_div(W * P, 8)
        nc.scalar.dma_start(out=recip_t[:, :rstep], in_=recip_d[:, :rstep])
        wt_t = cpool.tile([in_f, out_f], bf16)
        nc.scalar.dma_start(out=wt_t[:], in_=wt_d[:])
        biasT_t = cpool.tile([P, 1], f32)
        nc.scalar.dma_start(out=biasT_t[:], in_=biasT_d[:])
        for r0 in range(rstep, W * P, rstep):
            r1 = min(r0 + rstep, W * P)
            nc.scalar.dma_start(out=recip_t[:, r0:r1], in_=recip_d[:, r0:r1])

        # ---- lazily-emitted gather blocks and S batches ----
        # Queue assignment is static per (stream, block parity) so each pool
        # tag's DMA semaphore lane stays on one SWDGE queue.
        g_tiles = {}

        single_packet = os.environ.get("K_SP") == "1"

        def get_g(s, b):
            # One 2048-idx call per block, queue pinned to (stream, block
            # parity). Tag (s, b%2) rotates two slots, so blocks b and b+2
            # are the only in-flight gathers of their parity: each queue
            # holds at most 2 outstanding calls == its 2 DMASW lanes, and
            # the SWDGE descriptor ring never overflows even with the
            # Pool-engine DMASW waits stripped (see below). b+4 reuses b's
            # slot and so waits on b's consumers (=> b's DMA done).
            if (s, b) not in g_tiles:
                gt = gpool[s].tile([P, BLK * in_f], bf16, name=f"G{s}{b}",
                                   tag=f"G{s}{b % 2}")
                c0 = b * BLK
                nch = min(BLK, max(plan.Csum[s] - c0, 0))
                if skip_gather:
                    nc.vector.memset(gt[:], 0.0)
                elif nch > 0:
                    nc.gpsimd.dma_gather(
                        gt[:, :nch * in_f]
                        .rearrange("p (c e) -> p c e", e=in_f),
                        x_base[s],
                        gidx_t[s][:, c0 * P // 16:(c0 + nch) * P // 16],
                        nch * P,
                        nch * P,
                        in_f,
                        single_packet=single_packet,
                        queue_num=2 * (s == "hi") + (b % 2),
                    )
                g_tiles[(s, b)] = gt
            return g_tiles[(s, b)]

        s_tiles = {}

        def get_s(s, sb):
            if (s, sb) not in s_tiles:
                st = spool[s].tile([P, SBATCH * P], bf16, name=f"S{s}{sb}",
                                   tag=f"S{s}")
                nb = min(SBATCH, plan.Csum[s] - sb * SBATCH)
                in0 = crel_f[s][:, sb * SBATCH:sb * SBATCH + nb] \
                    .to_broadcast([P, nb, P])
                in1 = iota_t[:][:, None, :].to_broadcast([P, nb, P])
                outv = st[:].rearrange("p (b j) -> p b j", j=P)[:, :nb, :]
                nc.vector.tensor_tensor(out=outv, in0=in0, in1=in1,
                                        op=mybir.AluOpType.is_equal)
                s_tiles[(s, sb)] = st
            return s_tiles[(s, sb)]

        # ---- pre-issue every gather call, interleaved by stream progress,
        # so the Pool engine always has ready calls on all 4 queues ----
        order = sorted(
            [(s, b) for s in ("lo", "hi") for b in range(plan.NB[s])],
            key=lambda sb: (sb[1] + 0.5) / plan.NB[sb[0]])
        for s, b in order:
            get_g(s, b)

        if skip_compute:
            # touch each G tile minimally so gathers aren't dead-code'd
            acc = epool.tile([P, 1], f32, tag="acc")
            nc.vector.memset(acc[:], 0.0)
            for (s_, b_), gt in g_tiles.items():
                nc.vector.tensor_tensor(
                    out=acc[:], in0=acc[:], in1=gt[:, :2].bitcast(f32),
                    op=mybir.AluOpType.add)
            nc.sync.dma_start(out=outT_d[:1, :1], in_=acc[:1, :])

        # ---- main window loop ----
        for w in range(0 if skip_compute else W):
            chunks = []
            for s in ("lo", "hi"):
                chunks += [(s, c) for c in
                           range(plan.off[s][w], plan.off[s][w + 1])]
            # aggT[feat, dest] += G_chunk.T @ S_chunk
            psum_aggT = apool.tile([P, P], f32, tag="agg", name=f"agg{w}")
            n = len(chunks)
            for i, (s, c) in enumerate(chunks):
                b, slot = divmod(c, BLK)
                sb, ssub = divmod(c, SBATCH)
                gt = get_g(s, b)
                st = get_s(s, sb)
                nc.tensor.matmul(
                    out=psum_aggT[:],
                    lhsT=gt[:, slot * in_f:(slot + 1) * in_f],
                    rhs=st[:, ssub * P:(ssub + 1) * P],
                    start=(i == 0), stop=(i == n - 1))

            # hT[feat, dest] = aggT * recip_deg[dest]  (DVE: PSUM->SBUF)
            hT = epool.tile([P, P], bf16, tag="hT", name=f"hT{w}")
            nc.vector.tensor_tensor(out=hT[:], in0=psum_aggT[:],
                                    in1=recip_t[:, w * P:(w + 1) * P],
                                    op=mybir.AluOpType.mult)
            # outT[of, dest] = wt.T @ hT   (stationary wt)
            outp = opool.tile([P, P], f32, tag="outp", name=f"outp{w}")
            nc.tensor.matmul(out=outp[:], lhsT=wt_t[:], rhs=hT[:],
                             start=True, stop=True)
            # + bias: per-partition in transposed layout -> Scalar activation
            outs = epool.tile([P, P], f32, tag="outs", name=f"outs{w}")
            nc.scalar.add(out=outs[:], in_=outp[:], add=biasT_t[:])
            nc.sync.dma_start(out=outT_d[:, w * P:(w + 1) * P], in_=outs[:])

    # Strip Pool-engine waits on DMASW lane sems: they serialize each lane's
    # desc-gen behind the previous same-lane DMA *completion*, costing ~30us
    # of Pool-engine wait processing. Safe ONLY because get_g bounds each
    # queue to <=2 outstanding calls by construction (one per DMASW lane):
    # with more, the SWDGE descriptor ring is overwritten and gathers corrupt
    # (observed intermittently with round-robin queues). Consumers keep their
    # DMASW waits.
    for blk in nc.m.functions[0].blocks:
        for ins in blk.instructions:
            if ins.engine != mybir.EngineType.Pool:
                continue
            if not isinstance(ins, (mybir.InstDMAGatherAnt,
                                    mybir.InstEventSemaphore)):
                continue
            si = ins.sync_info
            if si is None or not si.on_wait:
                continue
            si.on_wait = [w for w in si.on_wait
                          if not (w.ant_name or "").startswith("DMASW")]

    nc.compile()
    return nc


def make_in_maps(plan, x, W, b):
    in_f = x.shape[1]
    out_f = W.shape[0]
    import ml_dtypes
    bf = ml_dtypes.bfloat16
    xb = np.ascontiguousarray(np.asarray(x, np.float32).astype(bf))
    base = {
        "xb": xb,
        "wt": np.ascontiguousarray(np.asarray(W, np.float32).T.astype(bf)),
        "biasT": np.ascontiguousarray(
            np.asarray(b, np.float32)[:, None]),
        "iota": np.tile(np.arange(P, dtype=np.float32)[None, :],
                        (P, 1)).astype(bf),
    }
    in_maps = []
    for k in range(plan.n_cores):
        m = dict(base)
        m["recip"] = np.ascontiguousarray(
            np.tile(plan.recip[k][None, :], (P, 1)))
        for name, arr in plan.core_arrays[k].items():
            s = name.split("_")[1]
            if plan.Csum[s] == 0:
                continue
            m[name] = arr
        in_maps.append(m)
    return in_maps


def run(x, edge_index, n_nodes, W, b, trace=False, trace_cores=None):
    from concourse.bass_utils import run_bass_kernel_spmd

    x = np.asarray(x)
    edge_index = np.asarray(edge_index)
    W = np.asarray(W)
    b = np.asarray(b)
    n_nodes = int(n_nodes)
    row = edge_index[0].astype(np.int64)
    col = edge_index[1].astype(np.int64)

    plan = Plan(row, col, n_nodes)
    nc = build_program(plan, x.shape[1], W.shape[0])
    in_maps = make_in_maps(plan, x, W, b)
    res = run_bass_kernel_spmd(nc, in_maps, core_ids=list(range(plan.n_cores)),
                               trace=trace, trace_cores=trace_cores)
    out = np.concatenate(
        [res.results[k]["outT"][:, :plan.d_core].T
         for k in range(plan.n_cores)], axis=0)
    return np.ascontiguousarray(out, dtype=np.float32), res


def kernel(x, edge_index, n_nodes, W, b):
    out, _ = run(x, edge_index, n_nodes, W, b)
    return out
